# revision 8
# baseline (speedup 1.0000x reference)
"""Trainium2 Bass kernel for nn_CausalTransformerEncoder_54142357733685.

6-layer transformer encoder (D=1024, H=16, DFF=4096, T=512 tokens) over a
batch of 16, data-parallel across 8 NeuronCores (2 batch elements per core).

Device-side design (per core, activations "feature-major" [feat, token]):
  - Residual stream xT kept fp32 in SBUF as [128, 8, 1024] (d-tile major,
    tokens = batch*512 + tok on the free axis).
  - LayerNorm statistics via ones-matmul partition reduction (sum and
    sum-of-squares replicated over partitions), rstd = Exp(-0.5*Ln(var+eps))
    on ScalarE (stays inside the exp/ln activation-table set).
  - All GEMMs bf16 with fp32 PSUM accumulation. LN gain/bias, the 1/sqrt(hd)
    score scale, and the V/out biases are folded into weights host-side.
  - Attention computed transposed: scoresT[tk,tq] = kT.T @ qT per
    (batch, head-pair), two heads row-packed in the PE array; softmax without
    max-subtraction (scores are small for this model family): exp on ScalarE,
    binary mask multiply on GpSimd, row sums + AV with both heads sharing one
    PSUM bank via half-ones / zero-padded-v stationary operands (all matmuls
    of the shared group write the full bank, so they form one WAW-ordered
    accumulation chain), normalization folded into the AV output.
  - FFN split into two independent f-halves per 512-token chunk so W1/W2/h
    tiles fit SBUF; each half's contribution is added to the residual.
"""

import math

import numpy as np
import ml_dtypes

import concourse.bass as bass
import concourse.tile as tile
from concourse import bacc, mybir
from concourse.bass import ts, ds
from concourse.bass_utils import run_bass_kernel_spmd

AFT = mybir.ActivationFunctionType
ALU = mybir.AluOpType
F32 = mybir.dt.float32
BF16 = mybir.dt.bfloat16
BF = ml_dtypes.bfloat16

B, N_TEXT, N_MOTION = 16, 64, 448
D, H, L, DFF, MAXSEQ = 1024, 16, 6, 4096, 512
HD = D // H
T = N_TEXT + N_MOTION  # 512
NCORES = 8
BL = B // NCORES  # 2 batch elements per core
TL = BL * T  # 1024 local tokens
DT = D // 128  # 8
FT = DFF // 128  # 32
EPS = 1e-5

_CACHE = {}


# --------------------------------------------------------------------------
# Device kernel builder
# --------------------------------------------------------------------------

def _build_nc(nlayers=L):
    nc = bacc.Bacc("TRN2", target_bir_lowering=False)

    x0 = nc.dram_tensor("x0", [DT, 128, TL], F32, kind="ExternalInput")
    bmask = nc.dram_tensor("bmask", [BL, 128, 4, T], BF16, kind="ExternalInput")
    wqk = nc.dram_tensor("wqk", [nlayers, 4, 128, 2, 2048], BF16, kind="ExternalInput")
    wv = nc.dram_tensor("wv", [nlayers, 2, 128, 4, 1024], BF16, kind="ExternalInput")
    wout = nc.dram_tensor("wout", [nlayers, 2, 128, 4, 1024], BF16, kind="ExternalInput")
    w1 = nc.dram_tensor("w1", [nlayers, 2, 4, 128, 2, 2048], BF16, kind="ExternalInput")
    w2 = nc.dram_tensor("w2", [nlayers, 2, 4, 128, 4, 1024], BF16, kind="ExternalInput")
    bqk = nc.dram_tensor("bqk", [nlayers, 128, 16], F32, kind="ExternalInput")
    bout = nc.dram_tensor("bout", [nlayers, 128, 8], F32, kind="ExternalInput")
    b1d = nc.dram_tensor("b1d", [nlayers, 128, 32], F32, kind="ExternalInput")
    b2d = nc.dram_tensor("b2d", [nlayers, 128, 8], F32, kind="ExternalInput")
    yout = nc.dram_tensor("yout", [DT, 128, TL], F32, kind="ExternalOutput")

    from contextlib import ExitStack

    with ExitStack() as ctx:
        tc = ctx.enter_context(tile.TileContext(nc))
        res_pool = ctx.enter_context(tc.tile_pool(name="res", bufs=1))
        const_pool = ctx.enter_context(tc.tile_pool(name="const", bufs=1))
        big_pool = ctx.enter_context(tc.tile_pool(name="big", bufs=1))
        w_pool = ctx.enter_context(tc.tile_pool(name="wp", bufs=7))
        cast_pool = ctx.enter_context(tc.tile_pool(name="cast", bufs=2))
        stat_pool = ctx.enter_context(tc.tile_pool(name="stat", bufs=3))
        ap_pool = ctx.enter_context(tc.tile_pool(name="appl", bufs=2))
        lr_pool = ctx.enter_context(tc.tile_pool(name="lrp", bufs=2))
        exp_pool = ctx.enter_context(tc.tile_pool(name="expp", bufs=2))
        mask_pool = ctx.enter_context(tc.tile_pool(name="maskp", bufs=2))
        bias_pool = ctx.enter_context(tc.tile_pool(name="biasp", bufs=2))
        ps_mm = ctx.enter_context(tc.tile_pool(name="psmm", bufs=2, space="PSUM"))
        ps_sc = ctx.enter_context(tc.tile_pool(name="pssc", bufs=2, space="PSUM"))
        ps_rs = ctx.enter_context(tc.tile_pool(name="psrs", bufs=1, space="PSUM"))
        ps_av = ctx.enter_context(tc.tile_pool(name="psav", bufs=1, space="PSUM"))
        if True:
            # constants
            ones_bf = const_pool.tile([128, 128], BF16, tag="ones")
            nc.vector.memset(ones_bf, 1.0)
            # half-ones for packed two-head reductions in one PSUM bank
            ones_a = const_pool.tile([128, 128], BF16, tag="ones_a")
            nc.vector.memset(ones_a, 0.0)
            nc.vector.memset(ones_a[:, 0:64], 1.0)
            ones_b = const_pool.tile([128, 128], BF16, tag="ones_b")
            nc.vector.memset(ones_b, 0.0)
            nc.vector.memset(ones_b[:, 64:128], 1.0)
            eps_t = const_pool.tile([128, 1], F32, tag="eps")
            nc.vector.memset(eps_t, EPS)

            # residual stream
            xT = res_pool.tile([128, DT, TL], F32, tag="xT")
            for dt in range(DT):
                nc.sync.dma_start(out=xT[:, dt, :], in_=x0[dt])

            # v stored zero-padded: per head 128 columns, first 64 = v,
            # last 64 = 0 -> head-pair AV matmuls can share one PSUM bank.
            vv = big_pool.tile([128, BL * 4, 16, 128], BF16, tag="vv")
            nc.vector.memset(vv[:], 0.0)

            def layernorm(dst):
                """dst <- bf16 feature-major standardized xT."""
                for tck in range(2):
                    psS = ps_mm.tile([128, 512], F32, tag="mm")
                    psQ = ps_mm.tile([128, 512], F32, tag="mm")
                    for dt in range(DT):
                        xb = cast_pool.tile([128, 512], BF16, tag="xb")
                        nc.vector.tensor_copy(out=xb[:], in_=xT[:, dt, ts(tck, 512)])
                        sq = cast_pool.tile([128, 512], BF16, tag="sq")
                        nc.scalar.activation(out=sq[:], in_=xT[:, dt, ts(tck, 512)],
                                             func=AFT.Square)
                        nc.tensor.matmul(psS[:], ones_bf[:], xb[:],
                                         start=(dt == 0), stop=(dt == DT - 1))
                        nc.tensor.matmul(psQ[:], ones_bf[:], sq[:],
                                         start=(dt == 0), stop=(dt == DT - 1))
                    # var*1024 = Q - S^2/1024  (tiles replicated over partitions)
                    sc_ = stat_pool.tile([128, 512], F32, tag="st")
                    nc.vector.tensor_copy(out=sc_[:], in_=psS[:])
                    st1 = stat_pool.tile([128, 512], F32, tag="st")
                    nc.vector.tensor_tensor(st1[:], sc_[:], psS[:], ALU.mult)
                    stv = stat_pool.tile([128, 512], F32, tag="st")
                    nc.vector.scalar_tensor_tensor(stv[:], st1[:], -1.0 / 1024.0,
                                                   psQ[:], ALU.mult, ALU.add)
                    # rstd = exp(-0.5 * ln(var + eps))
                    stl = stat_pool.tile([128, 512], F32, tag="st")
                    nc.scalar.activation(out=stl[:], in_=stv[:], func=AFT.Ln,
                                         scale=1.0 / 1024.0, bias=eps_t[:])
                    rstd = stat_pool.tile([128, 512], F32, tag="st")
                    nc.scalar.activation(out=rstd[:], in_=stl[:], func=AFT.Exp,
                                         scale=-0.5)
                    # mu*rstd = (S/1024) * rstd
                    mr = stat_pool.tile([128, 512], F32, tag="st")
                    nc.vector.scalar_tensor_tensor(mr[:], psS[:], 1.0 / 1024.0,
                                                   rstd[:], ALU.mult, ALU.mult)
                    for dt in range(DT):
                        tmp = ap_pool.tile([128, 512], F32, tag="ap")
                        nc.gpsimd.tensor_tensor(tmp[:], xT[:, dt, ts(tck, 512)],
                                                rstd[:], ALU.mult)
                        nc.vector.tensor_tensor(dst[:, dt, ts(tck, 512)], tmp[:],
                                                mr[:], ALU.subtract)

            for l in range(nlayers):
                # per-layer small tensors
                bqk_t = bias_pool.tile([128, 16], F32, tag="bqk")
                nc.sync.dma_start(out=bqk_t[:], in_=bqk[l])
                bout_t = bias_pool.tile([128, 8], F32, tag="bout")
                nc.sync.dma_start(out=bout_t[:], in_=bout[l])
                b1_t = bias_pool.tile([128, 32], F32, tag="b1")
                nc.sync.dma_start(out=b1_t[:], in_=b1d[l])
                b2_t = bias_pool.tile([128, 8], F32, tag="b2")
                nc.sync.dma_start(out=b2_t[:], in_=b2d[l])
                mk = []
                for b in range(BL):
                    m = mask_pool.tile([128, 4, T], BF16, tag="mask")
                    nc.sync.dma_start(out=m[:], in_=bmask[b])
                    mk.append(m)

                # ---------------- LN1 ----------------
                nrm = big_pool.tile([128, DT, TL], BF16, tag="nrm")
                layernorm(nrm)

                # ---------------- Q,K projections ----------------
                qT = big_pool.tile([128, 8, TL], BF16, tag="qT")
                kT = big_pool.tile([128, 8, TL], BF16, tag="kT")
                wqk_t = []
                for g in range(4):
                    wt = w_pool.tile([128, 2, 2048], BF16, tag="W", name=f"wqk{g}")
                    nc.sync.dma_start(out=wt[:], in_=wqk[l, g])
                    wqk_t.append(wt)
                for ot in range(16):
                    for tck in range(2):
                        ps = ps_mm.tile([128, 512], F32, tag="mm")
                        for dt in range(DT):
                            g, s = dt // 2, dt % 2
                            nc.tensor.matmul(ps[:], wqk_t[g][:, s, ts(ot, 128)],
                                             nrm[:, dt, ts(tck, 512)],
                                             start=(dt == 0), stop=(dt == DT - 1))
                        dst = qT if ot < 8 else kT
                        nc.vector.tensor_scalar_add(
                            out=dst[:, ot % 8, ts(tck, 512)], in0=ps[:],
                            scalar1=bqk_t[:, ot:ot + 1])

                # ---------------- V (token-major, zero-padded heads) --------
                wv_t = []
                for g in range(2):
                    wt = w_pool.tile([128, 4, 1024], BF16, tag="W", name=f"wv{g}")
                    nc.sync.dma_start(out=wt[:], in_=wv[l, g])
                    wv_t.append(wt)
                for tt in range(8):
                    for oc in range(2):
                        ps = ps_mm.tile([128, 512], F32, tag="mm")
                        for dt in range(DT):
                            g, s = dt // 4, dt % 4
                            nc.tensor.matmul(ps[:], nrm[:, dt, ts(tt, 128)],
                                             wv_t[g][:, s, ts(oc, 512)],
                                             start=(dt == 0), stop=(dt == DT - 1))
                        nc.vector.tensor_copy(
                            out=vv[:, tt, ds(oc * 8, 8), 0:64],
                            in_=ps[:].rearrange("p (h e) -> p h e", e=64))

                # ---------------- attention ----------------
                oT = big_pool.tile([128, 8, TL], BF16, tag="nrm")
                for b in range(BL):
                    for p in range(8):  # head pair
                        et = [exp_pool.tile([128, 4, T], BF16, tag="expt", name=f"et{h}")
                              for h in range(2)]
                        for kp in range(2):  # kt pairs (kt = 2*kp, 2*kp+1)
                            sc = [ps_sc.tile([128, 2, 512], F32, tag="sc", name=f"sc{h}")
                                  for h in range(2)]
                            for ki in range(2):
                                kt = 2 * kp + ki
                                for h in range(2):
                                    pr = ds(64 * h, 64)
                                    nc.tensor.matmul(
                                        sc[h][:, ki, :],
                                        kT[pr, p, ds(b * T + kt * 128, 128)],
                                        qT[pr, p, ds(b * T, T)],
                                        start=True, stop=True)
                            for h in range(2):
                                nc.scalar.activation(out=et[h][:, ds(2 * kp, 2), :],
                                                     in_=sc[h][:], func=AFT.Exp)
                                nc.gpsimd.tensor_tensor(
                                    et[h][:, ds(2 * kp, 2), :],
                                    et[h][:, ds(2 * kp, 2), :],
                                    mk[b][:, ds(2 * kp, 2), :], ALU.mult)
                        # row sums (heads packed in one bank via half-ones)
                        rs = ps_rs.tile([128, 512], F32, tag="rs")
                        first = True
                        for kt in range(4):
                            for h, oh in ((0, ones_a), (1, ones_b)):
                                nc.tensor.matmul(rs[:], oh[:], et[h][:, kt, :],
                                                 start=first,
                                                 stop=(kt == 3 and h == 1))
                                first = False
                        lnt = lr_pool.tile([128, 512], F32, tag="lns")
                        nc.scalar.activation(out=lnt[:], in_=rs[:], func=AFT.Ln)
                        riv = lr_pool.tile([128, 512], F32, tag="riv")
                        nc.scalar.activation(out=riv[:], in_=lnt[:], func=AFT.Exp,
                                             scale=-1.0)
                        # AV (heads packed via zero-padded v slices)
                        av = ps_av.tile([128, 512], F32, tag="av")
                        first = True
                        for kt in range(4):
                            for h in range(2):
                                lhs = vv[:, b * 4 + kt].rearrange(
                                    "p h e -> p (h e)")[:, ds(256 * p + 64 * h, 128)]
                                nc.tensor.matmul(av[:], lhs, et[h][:, kt, :],
                                                 start=first,
                                                 stop=(kt == 3 and h == 1))
                                first = False
                        nc.vector.tensor_tensor(oT[:, p, ds(b * T, T)], av[:],
                                                riv[:], ALU.mult)

                # ---------------- out projection + residual ----------------
                wout_t = []
                for g in range(2):
                    wt = w_pool.tile([128, 4, 1024], BF16, tag="W", name=f"wout{g}")
                    nc.sync.dma_start(out=wt[:], in_=wout[l, g])
                    wout_t.append(wt)
                for dt in range(DT):
                    for tck in range(2):
                        ps = ps_mm.tile([128, 512], F32, tag="mm")
                        for ot in range(8):
                            g, s = ot // 4, ot % 4
                            nc.tensor.matmul(ps[:], wout_t[g][:, s, ts(dt, 128)],
                                             oT[:, ot, ts(tck, 512)],
                                             start=(ot == 0), stop=(ot == 7))
                        nc.vector.scalar_tensor_tensor(
                            xT[:, dt, ts(tck, 512)], ps[:], bout_t[:, dt:dt + 1],
                            xT[:, dt, ts(tck, 512)], ALU.add, ALU.add)

                # ---------------- LN2 ----------------
                nrm2 = big_pool.tile([128, DT, TL], BF16, tag="nrm")
                layernorm(nrm2)

                # ---------------- FFN (per 512-token chunk, f-halves) -------
                for tck in range(2):
                    for fh in range(2):
                        w1_t = []
                        for g in range(4):
                            wt = w_pool.tile([128, 2, 2048], BF16, tag="W", name=f"w1_{g}")
                            nc.sync.dma_start(out=wt[:], in_=w1[l, fh, g])
                            w1_t.append(wt)
                        hts = [w_pool.tile([128, 8, 512], BF16, tag="W", name=f"hts{i}")
                               for i in range(2)]
                        for ft in range(16):
                            ps = ps_mm.tile([128, 512], F32, tag="mm")
                            for dt in range(DT):
                                g, s = dt // 2, dt % 2
                                nc.tensor.matmul(ps[:], w1_t[g][:, s, ts(ft, 128)],
                                                 nrm2[:, dt, ts(tck, 512)],
                                                 start=(dt == 0), stop=(dt == DT - 1))
                            ftg = fh * 16 + ft
                            nc.scalar.activation(out=hts[ft // 8][:, ft % 8, :],
                                                 in_=ps[:], func=AFT.Gelu,
                                                 bias=b1_t[:, ftg:ftg + 1])
                        w2_t = []
                        for g in range(4):
                            wt = w_pool.tile([128, 4, 1024], BF16, tag="W", name=f"w2_{g}")
                            nc.sync.dma_start(out=wt[:], in_=w2[l, fh, g])
                            w2_t.append(wt)
                        for dt in range(DT):
                            ps = ps_mm.tile([128, 512], F32, tag="mm")
                            for fi in range(16):
                                g, s = fi // 4, fi % 4
                                nc.tensor.matmul(ps[:], w2_t[g][:, s, ts(dt, 128)],
                                                 hts[fi // 8][:, fi % 8, :],
                                                 start=(fi == 0), stop=(fi == 15))
                            if fh == 0:
                                nc.vector.scalar_tensor_tensor(
                                    xT[:, dt, ts(tck, 512)], ps[:],
                                    b2_t[:, dt:dt + 1],
                                    xT[:, dt, ts(tck, 512)], ALU.add, ALU.add)
                            else:
                                nc.vector.tensor_tensor(
                                    xT[:, dt, ts(tck, 512)], ps[:],
                                    xT[:, dt, ts(tck, 512)], ALU.add)

            for dt in range(DT):
                nc.sync.dma_start(out=yout[dt], in_=xT[:, dt, :])

    nc.compile()
    return nc


# --------------------------------------------------------------------------
# Host-side input preparation
# --------------------------------------------------------------------------

def _combined_keep_maskT(n_text, pad):
    """Binary keep-mask, transposed: [B, k, q], 1.0 = may attend."""
    total = T
    can = np.zeros((total, total), dtype=bool)
    can[:n_text, :n_text] = True
    can[n_text:, :n_text] = True
    can[n_text:, n_text:] = np.tril(np.ones((N_MOTION, N_MOTION), dtype=bool))
    attn = ~can
    attn = np.broadcast_to(attn, (pad.shape[0], total, total)).copy()
    pad_full = np.concatenate(
        [np.zeros((pad.shape[0], n_text), dtype=bool), pad.astype(bool)], axis=1)
    attn |= pad_full[:, None, :]
    attn |= pad_full[:, :, None]
    attn &= ~np.eye(total, dtype=bool)[None]
    return (~attn).transpose(0, 2, 1).astype(np.float32)


def _prep_host(inputs):
    """Fold/transform all weights; returns dict of device arrays."""
    f = {k: np.asarray(v) for k, v in inputs.items()}
    n_text = int(f["n_text"])
    assert n_text == N_TEXT

    scale = 1.0 / math.sqrt(HD)
    dev = {}

    x = np.concatenate([f["text_tokens"], f["motion_tokens"]], axis=1)
    x = x.astype(np.float32) + f["pos_embed"][:T][None].astype(np.float32)
    dev["x_full"] = x  # [B, T, D]

    dev["keepT"] = _combined_keep_maskT(n_text, f["motion_padding_mask"])  # [B,k,q]

    wqk_l, wv_l, wout_l, w1_l, w2_l = [], [], [], [], []
    bqk_l, bout_l, b1_l, b2_l = [], [], [], []
    for l in range(L):
        g1 = f["ln1_g"][l].astype(np.float64)
        bg1 = f["ln1_b"][l].astype(np.float64)
        Wqkv = f["in_proj_w"][l].astype(np.float64)
        bqkv = f["in_proj_b"][l].astype(np.float64)
        Wq, Wk, Wv = Wqkv[:D], Wqkv[D:2 * D], Wqkv[2 * D:]
        Wq_f = Wq * g1[None, :] * scale
        Wk_f = Wk * g1[None, :]
        Wv_f = Wv * g1[None, :]
        bq_f = (bqkv[:D] + Wq @ bg1) * scale
        bk_f = bqkv[D:2 * D] + Wk @ bg1
        bv_f = bqkv[2 * D:] + Wv @ bg1
        Wo = f["out_w"][l].astype(np.float64)
        bo_f = f["out_b"][l].astype(np.float64) + Wo @ bv_f
        g2 = f["ln2_g"][l].astype(np.float64)
        bg2 = f["ln2_b"][l].astype(np.float64)
        W1 = f["ffn_w1"][l].astype(np.float64)
        W1_f = W1 * g2[None, :]
        b1_f = f["ffn_b1"][l].astype(np.float64) + W1 @ bg2
        W2 = f["ffn_w2"][l].astype(np.float64)
        b2_f = f["ffn_b2"][l].astype(np.float64)

        wqkT = np.concatenate([Wq_f, Wk_f], axis=0).T.astype(BF)  # [D, 2048]
        wqk_l.append(wqkT.reshape(4, 2, 128, 2048).transpose(0, 2, 1, 3))
        wvT = Wv_f.T.astype(BF)  # [D, 1024]
        wv_l.append(wvT.reshape(2, 4, 128, 1024).transpose(0, 2, 1, 3))
        woT = Wo.T.astype(BF)  # [o, do]
        wout_l.append(woT.reshape(2, 4, 128, 1024).transpose(0, 2, 1, 3))
        # w1T [D, DFF] -> [fh, g, 128, s, 2048]
        w1T = W1_f.T.astype(BF)
        w1_l.append(w1T.reshape(4, 2, 128, 2, 2048).transpose(3, 0, 2, 1, 4))
        # w2T [DFF, D] -> [fh, fg, 128, s, 1024]
        w2T = W2.T.astype(BF)
        w2_l.append(w2T.reshape(2, 4, 4, 128, 1024).transpose(0, 1, 3, 2, 4))

        bqk_l.append(np.concatenate([bq_f, bk_f]).astype(np.float32)
                     .reshape(16, 128).T)
        bout_l.append(bo_f.astype(np.float32).reshape(8, 128).T)
        b1_l.append(b1_f.astype(np.float32).reshape(32, 128).T)
        b2_l.append(b2_f.astype(np.float32).reshape(8, 128).T)

    dev["wqk"] = np.ascontiguousarray(np.stack(wqk_l))
    dev["wv"] = np.ascontiguousarray(np.stack(wv_l))
    dev["wout"] = np.ascontiguousarray(np.stack(wout_l))
    dev["w1"] = np.ascontiguousarray(np.stack(w1_l))
    dev["w2"] = np.ascontiguousarray(np.stack(w2_l))
    dev["bqk"] = np.ascontiguousarray(np.stack(bqk_l))
    dev["bout"] = np.ascontiguousarray(np.stack(bout_l))
    dev["b1d"] = np.ascontiguousarray(np.stack(b1_l))
    dev["b2d"] = np.ascontiguousarray(np.stack(b2_l))
    return dev


def _core_in_map(dev, c):
    xc = dev["x_full"][BL * c:BL * (c + 1)]  # [BL, T, D]
    x0 = xc.transpose(2, 0, 1).reshape(D, TL).reshape(DT, 128, TL)
    km = dev["keepT"][BL * c:BL * (c + 1)]  # [BL, k, q]
    bm = km.reshape(BL, 4, 128, T).transpose(0, 2, 1, 3).astype(BF)
    m = {
        "x0": np.ascontiguousarray(x0.astype(np.float32)),
        "bmask": np.ascontiguousarray(bm),
    }
    for k in ("wqk", "wv", "wout", "w1", "w2", "bqk", "bout", "b1d", "b2d"):
        m[k] = dev[k]
    return m


def _assemble(results):
    out = np.empty((B, T, D), dtype=np.float32)
    for c in range(NCORES):
        y = results[c]["yout"].reshape(D, TL)  # [d, b*T+t]
        out[BL * c:BL * (c + 1)] = y.reshape(D, BL, T).transpose(1, 2, 0)
    return out


def kernel(**inputs):
    if "nc" not in _CACHE:
        _CACHE["nc"] = _build_nc()
    nc = _CACHE["nc"]
    dev = _prep_host(inputs)
    in_maps = [_core_in_map(dev, c) for c in range(NCORES)]
    res = run_bass_kernel_spmd(nc, in_maps, core_ids=list(range(NCORES)))
    _CACHE["last"] = res
    return _assemble(res.results)


# revision 21
# speedup vs baseline: 1.4353x; 1.4353x over previous
"""Trainium2 Bass kernel for nn_CausalTransformerEncoder_54142357733685.

6-layer transformer encoder (D=1024, H=16, DFF=4096, T=512 tokens) over a
batch of 16, data-parallel across 8 NeuronCores (2 batch elements per core).

Device-side design (per core, activations "feature-major" [feat, token]):
  - Residual stream xT kept fp32 in SBUF as [128, 8, 1024] (d-tile major,
    tokens = batch*512 + tok on the free axis).
  - LayerNorm statistics via ones-matmul partition reduction (sum and
    sum-of-squares replicated over partitions), rstd = Exp(-0.5*Ln(var+eps))
    on ScalarE (stays inside the exp/ln activation-table set).
  - All GEMMs bf16 with fp32 PSUM accumulation. LN gain/bias, the 1/sqrt(hd)
    score scale, and the V/out biases are folded into weights host-side.
  - Attention computed transposed: scoresT[tk,tq] = kT.T @ qT per
    (batch, head-pair), two heads row-packed in the PE array; softmax without
    max-subtraction (scores are small for this model family): exp on ScalarE,
    binary mask multiply on GpSimd, row sums + AV with both heads sharing one
    PSUM bank via half-ones / zero-padded-v stationary operands (all matmuls
    of the shared group write the full bank, so they form one WAW-ordered
    accumulation chain), normalization folded into the AV output.
  - FFN split into two independent f-halves per 512-token chunk so W1/W2/h
    tiles fit SBUF; each half's contribution is added to the residual.
"""

import math

import numpy as np
import ml_dtypes

import concourse.bass as bass
import concourse.tile as tile
from concourse import bacc, mybir
from concourse.bass import ts, ds
from concourse.bass_utils import run_bass_kernel_spmd

AFT = mybir.ActivationFunctionType
ALU = mybir.AluOpType
F32 = mybir.dt.float32
BF16 = mybir.dt.bfloat16
BF = ml_dtypes.bfloat16

B, N_TEXT, N_MOTION = 16, 64, 448
D, H, L, DFF, MAXSEQ = 1024, 16, 6, 4096, 512
HD = D // H
T = N_TEXT + N_MOTION  # 512
NCORES = 8
BL = B // NCORES  # 2 batch elements per core
TL = BL * T  # 1024 local tokens
DT = D // 128  # 8
FT = DFF // 128  # 32
EPS = 1e-5
# compact attention layout: per key-chunk kt, only queries q >= 128*kt live
AOFF = (0, 512, 896, 1152)
AW = (512, 384, 256, 128)
ACOLS = 1280

_CACHE = {}


class _Bacc(bacc.Bacc):
    """Bacc with activation-table-set selection pinned so Ln/Exp/Square all
    resolve to natural_log_exp_and_others (one resident set for layernorm
    and softmax) instead of thrashing between exp_and_others / natural_log
    on every rowsum reciprocal."""

    def insert_act_table_loads(self):
        import bass_rust as _bass_rust
        from concourse.hw_specs import get_activation_tables

        has_activation = any(
            isinstance(i, mybir.InstActivation)
            for b in self.main_func.blocks
            for i in b.instructions
        )
        if not has_activation:
            return
        strip = {"Exp", "Ln", "Square"}
        tables = []
        for name, fns in get_activation_tables(self.m.arch).items():
            if name == "natural_log_exp_and_others":
                tables.append((name, set(fns)))
            else:
                tables.append(
                    (name, {f for f in fns if getattr(f, "name", str(f)) not in strip}))
        _bass_rust.insert_act_table_loads(self, tables)


# --------------------------------------------------------------------------
# Device kernel builder
# --------------------------------------------------------------------------

def _build_nc(nlayers=L):
    nc = _Bacc("TRN2", target_bir_lowering=False)

    x0 = nc.dram_tensor("x0", [DT, 128, TL], F32, kind="ExternalInput")
    bmask = nc.dram_tensor("bmask", [BL, 128, 1280], BF16, kind="ExternalInput")
    wqk = nc.dram_tensor("wqk", [nlayers, 4, 128, 2, 2048], BF16, kind="ExternalInput")
    wv = nc.dram_tensor("wv", [nlayers, 2, 128, 4, 1024], BF16, kind="ExternalInput")
    wout = nc.dram_tensor("wout", [nlayers, 2, 128, 4, 1024], BF16, kind="ExternalInput")
    w1 = nc.dram_tensor("w1", [nlayers, 2, 4, 128, 2, 2048], BF16, kind="ExternalInput")
    w2 = nc.dram_tensor("w2", [nlayers, 2, 4, 128, 4, 1024], BF16, kind="ExternalInput")
    bqk = nc.dram_tensor("bqk", [nlayers, 128, 16], F32, kind="ExternalInput")
    bout = nc.dram_tensor("bout", [nlayers, 128, 8], F32, kind="ExternalInput")
    b1d = nc.dram_tensor("b1d", [nlayers, 128, 32], F32, kind="ExternalInput")
    b2d = nc.dram_tensor("b2d", [nlayers, 128, 8], F32, kind="ExternalInput")
    yout = nc.dram_tensor("yout", [DT, 128, TL], F32, kind="ExternalOutput")

    from contextlib import ExitStack

    with ExitStack() as ctx:
        tc = ctx.enter_context(tile.TileContext(nc))
        res_pool = ctx.enter_context(tc.tile_pool(name="res", bufs=1))
        const_pool = ctx.enter_context(tc.tile_pool(name="const", bufs=1))
        big_pool = ctx.enter_context(tc.tile_pool(name="big", bufs=1))
        w_pool = ctx.enter_context(tc.tile_pool(name="wp", bufs=8))
        cast_pool = ctx.enter_context(tc.tile_pool(name="cast", bufs=2))
        stat_pool = ctx.enter_context(tc.tile_pool(name="stat", bufs=3))
        ap_pool = ctx.enter_context(tc.tile_pool(name="appl", bufs=2))
        lr_pool = ctx.enter_context(tc.tile_pool(name="lrp", bufs=2))
        exp_pool = ctx.enter_context(tc.tile_pool(name="expp", bufs=3))
        mask_pool = ctx.enter_context(tc.tile_pool(name="maskp", bufs=2))
        bias_pool = ctx.enter_context(tc.tile_pool(name="biasp", bufs=2))
        ps_mm = ctx.enter_context(tc.tile_pool(name="psmm", bufs=2, space="PSUM"))
        ps_sc = ctx.enter_context(tc.tile_pool(name="pssc", bufs=3, space="PSUM"))
        ps_rs = ctx.enter_context(tc.tile_pool(name="psrs", bufs=2, space="PSUM"))
        ps_av = ctx.enter_context(tc.tile_pool(name="psav", bufs=1, space="PSUM"))
        if True:
            # constants
            ones_bf = const_pool.tile([128, 128], BF16, tag="ones")
            nc.vector.memset(ones_bf, 1.0)
            # half-ones for packed two-head reductions in one PSUM bank
            ones_a = const_pool.tile([128, 128], BF16, tag="ones_a")
            nc.vector.memset(ones_a, 0.0)
            nc.vector.memset(ones_a[:, 0:64], 1.0)
            ones_b = const_pool.tile([128, 128], BF16, tag="ones_b")
            nc.vector.memset(ones_b, 0.0)
            nc.vector.memset(ones_b[:, 64:128], 1.0)
            eps_t = const_pool.tile([128, 1], F32, tag="eps")
            nc.vector.memset(eps_t, EPS)

            # residual stream
            xT = res_pool.tile([128, DT, TL], F32, tag="xT")
            for dt in range(DT):
                nc.sync.dma_start(out=xT[:, dt, :], in_=x0[dt])

            # v stored zero-padded: per head 128 columns, first 64 = v,
            # last 64 = 0 -> head-pair AV matmuls can share one PSUM bank.
            vv = big_pool.tile([128, BL * 4, 16, 128], BF16, tag="vv")
            nc.vector.memset(vv[:], 0.0)

            def layernorm_chunk(dst, tck, pool_casts=False):
                """dst[:, :, tck*512:+512] <- bf16 standardized xT chunk."""
                psS = ps_mm.tile([128, 512], F32, tag="mm", name="psS")
                psQ = ps_mm.tile([128, 512], F32, tag="mm", name="psQ")
                for dt in range(DT):
                    xb = cast_pool.tile([128, 512], BF16, tag="xb", name="xb")
                    sq = cast_pool.tile([128, 512], BF16, tag="sq", name="sq")
                    if pool_casts:
                        nc.gpsimd.tensor_copy(out=xb[:], in_=xT[:, dt, ts(tck, 512)])
                        nc.gpsimd.tensor_tensor(sq[:], xT[:, dt, ts(tck, 512)],
                                                xT[:, dt, ts(tck, 512)], ALU.mult)
                    else:
                        nc.scalar.copy(out=xb[:], in_=xT[:, dt, ts(tck, 512)])
                        nc.vector.tensor_tensor(sq[:], xT[:, dt, ts(tck, 512)],
                                                xT[:, dt, ts(tck, 512)], ALU.mult)
                    nc.tensor.matmul(psS[:], ones_bf[:], xb[:],
                                     start=(dt == 0), stop=(dt == DT - 1))
                    nc.tensor.matmul(psQ[:], ones_bf[:], sq[:],
                                     start=(dt == 0), stop=(dt == DT - 1))
                # var*1024 = Q - S^2/1024  (tiles replicated over partitions)
                sc_ = stat_pool.tile([128, 512], F32, tag="st", name="sc_")
                nc.vector.tensor_copy(out=sc_[:], in_=psS[:])
                st1 = stat_pool.tile([128, 512], F32, tag="st", name="st1")
                nc.vector.tensor_tensor(st1[:], sc_[:], psS[:], ALU.mult)
                stv = stat_pool.tile([128, 512], F32, tag="st", name="stv")
                nc.vector.scalar_tensor_tensor(stv[:], st1[:], -1.0 / 1024.0,
                                               psQ[:], ALU.mult, ALU.add)
                # rstd = exp(-0.5 * ln(var + eps))
                stl = stat_pool.tile([128, 512], F32, tag="st", name="stl")
                nc.scalar.activation(out=stl[:], in_=stv[:], func=AFT.Ln,
                                     scale=1.0 / 1024.0, bias=eps_t[:])
                rstd = stat_pool.tile([128, 512], F32, tag="st", name="rstd")
                nc.scalar.activation(out=rstd[:], in_=stl[:], func=AFT.Exp,
                                     scale=-0.5)
                # mu*rstd = (S/1024) * rstd
                mr = stat_pool.tile([128, 512], F32, tag="st", name="mr")
                nc.vector.scalar_tensor_tensor(mr[:], psS[:], 1.0 / 1024.0,
                                               rstd[:], ALU.mult, ALU.mult)
                for dt in range(DT):
                    tmp = ap_pool.tile([128, 512], F32, tag="ap", name="tmp")
                    nc.gpsimd.tensor_tensor(tmp[:], xT[:, dt, ts(tck, 512)],
                                            rstd[:], ALU.mult)
                    nc.vector.tensor_tensor(dst[:, dt, ts(tck, 512)], tmp[:],
                                            mr[:], ALU.subtract)

            for l in range(nlayers):
                # per-layer small tensors
                bqk_t = bias_pool.tile([128, 16], F32, tag="bqk")
                nc.sync.dma_start(out=bqk_t[:], in_=bqk[l])
                bout_t = bias_pool.tile([128, 8], F32, tag="bout")
                nc.sync.dma_start(out=bout_t[:], in_=bout[l])
                b1_t = bias_pool.tile([128, 32], F32, tag="b1")
                nc.sync.dma_start(out=b1_t[:], in_=b1d[l])
                b2_t = bias_pool.tile([128, 8], F32, tag="b2")
                nc.sync.dma_start(out=b2_t[:], in_=b2d[l])
                mk = []
                for b in range(BL):
                    m = mask_pool.tile([128, ACOLS], BF16, tag="mask", name=f"mk{b}")
                    nc.sync.dma_start(out=m[:], in_=bmask[b])
                    mk.append(m)

                # ---------------- LN1 ----------------
                nrm = big_pool.tile([128, DT, TL], BF16, tag="nrm")
                layernorm_chunk(nrm, 0)
                layernorm_chunk(nrm, 1)

                # ---------------- Q,K projections (tck-outer) ----------------
                qT = big_pool.tile([128, 8, TL], BF16, tag="qT")
                kT = big_pool.tile([128, 8, TL], BF16, tag="kT")
                wqk_t = []
                for g in range(4):
                    wt = w_pool.tile([128, 2, 2048], BF16, tag="W", name=f"wqk{g}")
                    nc.sync.dma_start(out=wt[:, 0], in_=wqk[l, g, :, 0])
                    nc.sync.dma_start(out=wt[:, 1], in_=wqk[l, g, :, 1])
                    wqk_t.append(wt)
                for tck in range(2):
                    for ot in range(16):
                        ps = ps_mm.tile([128, 512], F32, tag="mm", name="psqk")
                        for dt in range(DT):
                            g, s = dt // 2, dt % 2
                            nc.tensor.matmul(ps[:], wqk_t[g][:, s, ts(ot, 128)],
                                             nrm[:, dt, ts(tck, 512)],
                                             start=(dt == 0), stop=(dt == DT - 1))
                        dst = qT if ot < 8 else kT
                        nc.vector.tensor_scalar_add(
                            out=dst[:, ot % 8, ts(tck, 512)], in0=ps[:],
                            scalar1=bqk_t[:, ot:ot + 1])

                # ---------------- V (token-major, zero-padded heads) --------
                wv_t = []
                for g in range(2):
                    wt = w_pool.tile([128, 4, 1024], BF16, tag="W", name=f"wv{g}")
                    nc.sync.dma_start(out=wt[:, 0:2], in_=wv[l, g, :, 0:2])
                    nc.sync.dma_start(out=wt[:, 2:4], in_=wv[l, g, :, 2:4])
                    wv_t.append(wt)
                for tt in range(8):
                    for oc in range(2):
                        ps = ps_mm.tile([128, 512], F32, tag="mm", name="psv")
                        for dt in range(DT):
                            g, s = dt // 4, dt % 4
                            nc.tensor.matmul(ps[:], nrm[:, dt, ts(tt, 128)],
                                             wv_t[g][:, s, ts(oc, 512)],
                                             start=(dt == 0), stop=(dt == DT - 1))
                        nc.vector.tensor_copy(
                            out=vv[:, tt, ds(oc * 8, 8), 0:64],
                            in_=ps[:].rearrange("p (h e) -> p h e", e=64))

                # ---------------- attention + out-projection ----------------
                # causal structure: key chunk kt (kt>=1) only reaches queries
                # q >= 128*kt, so score/rowsum/AV matmuls skip the dead range.
                oT = big_pool.tile([128, 8, TL], BF16, tag="nrm")
                wout_t = []

                def attention_batch(b):
                    for p in range(8):  # head pair
                        et = [exp_pool.tile([128, ACOLS], BF16, tag="expt",
                                            name=f"et{h}") for h in range(2)]
                        for kt in range(4):
                            q0 = 128 * kt
                            sc = [ps_sc.tile([128, 512], F32, tag="sc",
                                             name=f"sc{h}") for h in range(2)]
                            for h in range(2):
                                pr = ds(64 * h, 64)
                                nc.tensor.matmul(
                                    sc[h][:, q0:],
                                    kT[pr, p, ds(b * T + kt * 128, 128)],
                                    qT[pr, p, ds(b * T + q0, T - q0)],
                                    start=True, stop=True)
                            for h in range(2):
                                nc.scalar.activation(
                                    out=et[h][:, ds(AOFF[kt], AW[kt])],
                                    in_=sc[h][:, q0:], func=AFT.Exp)
                        for h in range(2):
                            nc.vector.tensor_tensor(et[h][:], et[h][:], mk[b][:],
                                                    ALU.mult)
                        # row sums (heads packed in one bank via half-ones)
                        rs = ps_rs.tile([128, 512], F32, tag="rs")
                        first = True
                        for h, oh in ((0, ones_a), (1, ones_b)):
                            for kt in range(4):
                                q0 = 128 * kt
                                nc.tensor.matmul(rs[:, q0:], oh[:],
                                                 et[h][:, ds(AOFF[kt], AW[kt])],
                                                 start=first,
                                                 stop=(kt == 3 and h == 1))
                                first = False
                        lnt = stat_pool.tile([128, 512], F32, tag="st", name="lnt")
                        nc.scalar.activation(out=lnt[:], in_=rs[:], func=AFT.Ln)
                        riv = lr_pool.tile([128, 512], F32, tag="riv")
                        nc.scalar.activation(out=riv[:], in_=lnt[:], func=AFT.Exp,
                                             scale=-1.0)
                        # AV (heads packed via zero-padded v slices)
                        av = ps_av.tile([128, 512], F32, tag="av")
                        first = True
                        for h in range(2):
                            for kt in range(4):
                                q0 = 128 * kt
                                lhs = vv[:, b * 4 + kt].rearrange(
                                    "p h e -> p (h e)")[:, ds(256 * p + 64 * h, 128)]
                                nc.tensor.matmul(av[:, q0:], lhs,
                                                 et[h][:, ds(AOFF[kt], AW[kt])],
                                                 start=first,
                                                 stop=(kt == 3 and h == 1))
                                first = False
                        nc.vector.tensor_tensor(oT[:, p, ds(b * T, T)], av[:],
                                                riv[:], ALU.mult)

                def outproj_batch(b):
                    for dt in range(DT):
                        ps = ps_mm.tile([128, 512], F32, tag="mm", name="pso")
                        for ot in range(8):
                            g, s = ot // 4, ot % 4
                            nc.tensor.matmul(ps[:], wout_t[g][:, s, ts(dt, 128)],
                                             oT[:, ot, ts(b, 512)],
                                             start=(ot == 0), stop=(ot == 7))
                        nc.vector.scalar_tensor_tensor(
                            xT[:, dt, ts(b, 512)], ps[:], bout_t[:, dt:dt + 1],
                            xT[:, dt, ts(b, 512)], ALU.add, ALU.add)

                attention_batch(0)
                for g in range(2):
                    wt = w_pool.tile([128, 4, 1024], BF16, tag="W", name=f"wout{g}")
                    nc.sync.dma_start(out=wt[:, 0:2], in_=wout[l, g, :, 0:2])
                    nc.sync.dma_start(out=wt[:, 2:4], in_=wout[l, g, :, 2:4])
                    wout_t.append(wt)
                outproj_batch(0)
                attention_batch(1)
                outproj_batch(1)

                # ---------------- LN2 ----------------
                nrm2 = big_pool.tile([128, DT, TL], BF16, tag="nrm")
                layernorm_chunk(nrm2, 0)
                layernorm_chunk(nrm2, 1)

                # ---------------- FFN (f-half outer; W1/W2 loaded once) ------
                for fh in range(2):
                    w1_t = []
                    for g in range(4):
                        wt = w_pool.tile([128, 2, 2048], BF16, tag="W",
                                         name=f"w1_{g}")
                        nc.sync.dma_start(out=wt[:, 0], in_=w1[l, fh, g, :, 0])
                        nc.sync.dma_start(out=wt[:, 1], in_=w1[l, fh, g, :, 1])
                        w1_t.append(wt)
                    hts = {}
                    for tck in range(2):
                        hts[tck] = [w_pool.tile([128, 8, 512], BF16, tag="W",
                                                name=f"hts{tck}_{i}")
                                    for i in range(2)]
                        for ft in range(16):
                            ps = ps_mm.tile([128, 512], F32, tag="mm", name="psf1")
                            for dt in range(DT):
                                g, s = dt // 2, dt % 2
                                nc.tensor.matmul(ps[:], w1_t[g][:, s, ts(ft, 128)],
                                                 nrm2[:, dt, ts(tck, 512)],
                                                 start=(dt == 0), stop=(dt == DT - 1))
                            ftg = fh * 16 + ft
                            nc.scalar.activation(out=hts[tck][ft // 8][:, ft % 8, :],
                                                 in_=ps[:], func=AFT.Gelu,
                                                 bias=b1_t[:, ftg:ftg + 1])
                    w2_t = []
                    for g in range(4):
                        wt = w_pool.tile([128, 4, 1024], BF16, tag="W",
                                         name=f"w2_{g}")
                        nc.sync.dma_start(out=wt[:, 0:2], in_=w2[l, fh, g, :, 0:2])
                        nc.sync.dma_start(out=wt[:, 2:4], in_=w2[l, fh, g, :, 2:4])
                        w2_t.append(wt)
                    for tck in range(2):
                        for dt in range(DT):
                            ps = ps_mm.tile([128, 512], F32, tag="mm", name="psf2")
                            for fi in range(16):
                                g, s = fi // 4, fi % 4
                                nc.tensor.matmul(ps[:], w2_t[g][:, s, ts(dt, 128)],
                                                 hts[tck][fi // 8][:, fi % 8, :],
                                                 start=(fi == 0), stop=(fi == 15))
                            if fh == 0:
                                nc.vector.scalar_tensor_tensor(
                                    xT[:, dt, ts(tck, 512)], ps[:],
                                    b2_t[:, dt:dt + 1],
                                    xT[:, dt, ts(tck, 512)], ALU.add, ALU.add)
                            else:
                                nc.vector.tensor_tensor(
                                    xT[:, dt, ts(tck, 512)], ps[:],
                                    xT[:, dt, ts(tck, 512)], ALU.add)

            for dt in range(DT):
                nc.sync.dma_start(out=yout[dt], in_=xT[:, dt, :])

    nc.compile()
    return nc


# --------------------------------------------------------------------------
# Host-side input preparation
# --------------------------------------------------------------------------

def _combined_keep_maskT(n_text, pad):
    """Binary keep-mask, transposed: [B, k, q], 1.0 = may attend."""
    total = T
    can = np.zeros((total, total), dtype=bool)
    can[:n_text, :n_text] = True
    can[n_text:, :n_text] = True
    can[n_text:, n_text:] = np.tril(np.ones((N_MOTION, N_MOTION), dtype=bool))
    attn = ~can
    attn = np.broadcast_to(attn, (pad.shape[0], total, total)).copy()
    pad_full = np.concatenate(
        [np.zeros((pad.shape[0], n_text), dtype=bool), pad.astype(bool)], axis=1)
    attn |= pad_full[:, None, :]
    attn |= pad_full[:, :, None]
    attn &= ~np.eye(total, dtype=bool)[None]
    return (~attn).transpose(0, 2, 1).astype(np.float32)


def _prep_host(inputs):
    """Fold/transform all weights; returns dict of device arrays."""
    f = {k: np.asarray(v) for k, v in inputs.items()}
    n_text = int(f["n_text"])
    assert n_text == N_TEXT

    scale = 1.0 / math.sqrt(HD)
    dev = {}

    x = np.concatenate([f["text_tokens"], f["motion_tokens"]], axis=1)
    x = x.astype(np.float32) + f["pos_embed"][:T][None].astype(np.float32)
    dev["x_full"] = x  # [B, T, D]

    dev["keepT"] = _combined_keep_maskT(n_text, f["motion_padding_mask"])  # [B,k,q]

    wqk_l, wv_l, wout_l, w1_l, w2_l = [], [], [], [], []
    bqk_l, bout_l, b1_l, b2_l = [], [], [], []
    for l in range(L):
        g1 = f["ln1_g"][l].astype(np.float64)
        bg1 = f["ln1_b"][l].astype(np.float64)
        Wqkv = f["in_proj_w"][l].astype(np.float64)
        bqkv = f["in_proj_b"][l].astype(np.float64)
        Wq, Wk, Wv = Wqkv[:D], Wqkv[D:2 * D], Wqkv[2 * D:]
        Wq_f = Wq * g1[None, :] * scale
        Wk_f = Wk * g1[None, :]
        Wv_f = Wv * g1[None, :]
        bq_f = (bqkv[:D] + Wq @ bg1) * scale
        bk_f = bqkv[D:2 * D] + Wk @ bg1
        bv_f = bqkv[2 * D:] + Wv @ bg1
        Wo = f["out_w"][l].astype(np.float64)
        bo_f = f["out_b"][l].astype(np.float64) + Wo @ bv_f
        g2 = f["ln2_g"][l].astype(np.float64)
        bg2 = f["ln2_b"][l].astype(np.float64)
        W1 = f["ffn_w1"][l].astype(np.float64)
        W1_f = W1 * g2[None, :]
        b1_f = f["ffn_b1"][l].astype(np.float64) + W1 @ bg2
        W2 = f["ffn_w2"][l].astype(np.float64)
        b2_f = f["ffn_b2"][l].astype(np.float64)

        wqkT = np.concatenate([Wq_f, Wk_f], axis=0).T.astype(BF)  # [D, 2048]
        wqk_l.append(wqkT.reshape(4, 2, 128, 2048).transpose(0, 2, 1, 3))
        wvT = Wv_f.T.astype(BF)  # [D, 1024]
        wv_l.append(wvT.reshape(2, 4, 128, 1024).transpose(0, 2, 1, 3))
        woT = Wo.T.astype(BF)  # [o, do]
        wout_l.append(woT.reshape(2, 4, 128, 1024).transpose(0, 2, 1, 3))
        # w1T [D, DFF] -> [fh, g, 128, s, 2048]
        w1T = W1_f.T.astype(BF)
        w1_l.append(w1T.reshape(4, 2, 128, 2, 2048).transpose(3, 0, 2, 1, 4))
        # w2T [DFF, D] -> [fh, fg, 128, s, 1024]
        w2T = W2.T.astype(BF)
        w2_l.append(w2T.reshape(2, 4, 4, 128, 1024).transpose(0, 1, 3, 2, 4))

        bqk_l.append(np.concatenate([bq_f, bk_f]).astype(np.float32)
                     .reshape(16, 128).T)
        bout_l.append(bo_f.astype(np.float32).reshape(8, 128).T)
        b1_l.append(b1_f.astype(np.float32).reshape(32, 128).T)
        b2_l.append(b2_f.astype(np.float32).reshape(8, 128).T)

    dev["wqk"] = np.ascontiguousarray(np.stack(wqk_l))
    dev["wv"] = np.ascontiguousarray(np.stack(wv_l))
    dev["wout"] = np.ascontiguousarray(np.stack(wout_l))
    dev["w1"] = np.ascontiguousarray(np.stack(w1_l))
    dev["w2"] = np.ascontiguousarray(np.stack(w2_l))
    dev["bqk"] = np.ascontiguousarray(np.stack(bqk_l))
    dev["bout"] = np.ascontiguousarray(np.stack(bout_l))
    dev["b1d"] = np.ascontiguousarray(np.stack(b1_l))
    dev["b2d"] = np.ascontiguousarray(np.stack(b2_l))
    return dev


AOFF = (0, 512, 896, 1152)
AW = (512, 384, 256, 128)


def _core_in_map(dev, c):
    xc = dev["x_full"][BL * c:BL * (c + 1)]  # [BL, T, D]
    x0 = xc.transpose(2, 0, 1).reshape(D, TL).reshape(DT, 128, TL)
    km = dev["keepT"][BL * c:BL * (c + 1)]  # [BL, k, q]
    kc = km.reshape(BL, 4, 128, T)
    bm = np.zeros((BL, 128, ACOLS), dtype=BF)
    for kt in range(4):
        bm[:, :, AOFF[kt]:AOFF[kt] + AW[kt]] = kc[:, kt, :, 128 * kt:].astype(BF)
    m = {
        "x0": np.ascontiguousarray(x0.astype(np.float32)),
        "bmask": np.ascontiguousarray(bm),
    }
    for k in ("wqk", "wv", "wout", "w1", "w2", "bqk", "bout", "b1d", "b2d"):
        m[k] = dev[k]
    return m


def _assemble(results):
    out = np.empty((B, T, D), dtype=np.float32)
    for c in range(NCORES):
        y = results[c]["yout"].reshape(D, TL)  # [d, b*T+t]
        out[BL * c:BL * (c + 1)] = y.reshape(D, BL, T).transpose(1, 2, 0)
    return out


def kernel(**inputs):
    if "nc" not in _CACHE:
        _CACHE["nc"] = _build_nc()
    nc = _CACHE["nc"]
    dev = _prep_host(inputs)
    in_maps = [_core_in_map(dev, c) for c in range(NCORES)]
    res = run_bass_kernel_spmd(nc, in_maps, core_ids=list(range(NCORES)))
    _CACHE["last"] = res
    return _assemble(res.results)


# revision 32
# speedup vs baseline: 1.5067x; 1.0498x over previous
"""Trainium2 Bass kernel for nn_CausalTransformerEncoder_54142357733685.

6-layer transformer encoder (D=1024, H=16, DFF=4096, T=512 tokens) over a
batch of 16, data-parallel across 8 NeuronCores (2 batch elements per core).

Device-side design (per core, activations "feature-major" [feat, token]):
  - Residual stream xT kept fp32 in SBUF as [128, 8, 1024] (d-tile major,
    tokens = batch*512 + tok on the free axis).
  - LayerNorm statistics via ones-matmul partition reduction (sum and
    sum-of-squares replicated over partitions), rstd = Exp(-0.5*Ln(var+eps))
    on ScalarE (stays inside the exp/ln activation-table set).
  - All GEMMs bf16 with fp32 PSUM accumulation. LN gain/bias, the 1/sqrt(hd)
    score scale, and the V/out biases are folded into weights host-side.
  - Attention computed transposed: scoresT[tk,tq] = kT.T @ qT per
    (batch, head-pair), two heads row-packed in the PE array; softmax without
    max-subtraction (scores are small for this model family): exp on ScalarE,
    binary mask multiply on GpSimd, row sums + AV with both heads sharing one
    PSUM bank via half-ones / zero-padded-v stationary operands (all matmuls
    of the shared group write the full bank, so they form one WAW-ordered
    accumulation chain), normalization folded into the AV output.
  - FFN split into two independent f-halves per 512-token chunk so W1/W2/h
    tiles fit SBUF; each half's contribution is added to the residual.
"""

import math

import numpy as np
import ml_dtypes

import concourse.bass as bass
import concourse.tile as tile
from concourse import bacc, mybir
from concourse.bass import ts, ds
from concourse.bass_utils import run_bass_kernel_spmd

AFT = mybir.ActivationFunctionType
ALU = mybir.AluOpType
F32 = mybir.dt.float32
BF16 = mybir.dt.bfloat16
BF = ml_dtypes.bfloat16

B, N_TEXT, N_MOTION = 16, 64, 448
D, H, L, DFF, MAXSEQ = 1024, 16, 6, 4096, 512
HD = D // H
T = N_TEXT + N_MOTION  # 512
NCORES = 8
BL = B // NCORES  # 2 batch elements per core
TL = BL * T  # 1024 local tokens
DT = D // 128  # 8
FT = DFF // 128  # 32
EPS = 1e-5
# compact attention layout: per key-chunk kt, only queries q >= 128*kt live
AOFF = (0, 512, 896, 1152)
AW = (512, 384, 256, 128)
ACOLS = 1280

_CACHE = {}


class _Bacc(bacc.Bacc):
    """Bacc with activation-table-set selection pinned so Ln/Exp/Square all
    resolve to natural_log_exp_and_others (one resident set for layernorm
    and softmax) instead of thrashing between exp_and_others / natural_log
    on every rowsum reciprocal."""

    def insert_act_table_loads(self):
        import bass_rust as _bass_rust
        from concourse.hw_specs import get_activation_tables

        has_activation = any(
            isinstance(i, mybir.InstActivation)
            for b in self.main_func.blocks
            for i in b.instructions
        )
        if not has_activation:
            return
        strip = {"Exp", "Ln", "Square"}
        tables = []
        for name, fns in get_activation_tables(self.m.arch).items():
            if name == "natural_log_exp_and_others":
                tables.append((name, set(fns)))
            else:
                tables.append(
                    (name, {f for f in fns if getattr(f, "name", str(f)) not in strip}))
        _bass_rust.insert_act_table_loads(self, tables)


# --------------------------------------------------------------------------
# Device kernel builder
# --------------------------------------------------------------------------

def _build_nc(nlayers=L):
    nc = _Bacc("TRN2", target_bir_lowering=False,
               dynamic_dma_scratch_size=2048)

    x0 = nc.dram_tensor("x0", [DT, 128, TL], F32, kind="ExternalInput")
    bmask = nc.dram_tensor("bmask", [BL, 128, 1280], BF16, kind="ExternalInput")
    wqk = nc.dram_tensor("wqk", [nlayers, 4, 128, 2, 2048], BF16, kind="ExternalInput")
    wv = nc.dram_tensor("wv", [nlayers, 2, 128, 4, 1024], BF16, kind="ExternalInput")
    wout = nc.dram_tensor("wout", [nlayers, 2, 128, 4, 1024], BF16, kind="ExternalInput")
    w1 = nc.dram_tensor("w1", [nlayers, 2, 4, 128, 2, 2048], BF16, kind="ExternalInput")
    w2 = nc.dram_tensor("w2", [nlayers, 2, 4, 128, 4, 1024], BF16, kind="ExternalInput")
    bqk = nc.dram_tensor("bqk", [nlayers, 128, 16], F32, kind="ExternalInput")
    bout = nc.dram_tensor("bout", [nlayers, 128, 8], F32, kind="ExternalInput")
    b1d = nc.dram_tensor("b1d", [nlayers, 128, 32], F32, kind="ExternalInput")
    b2d = nc.dram_tensor("b2d", [nlayers, 128, 8], F32, kind="ExternalInput")
    yout = nc.dram_tensor("yout", [DT, 128, TL], F32, kind="ExternalOutput")

    from contextlib import ExitStack

    with ExitStack() as ctx:
        tc = ctx.enter_context(tile.TileContext(nc))
        res_pool = ctx.enter_context(tc.tile_pool(name="res", bufs=1))
        const_pool = ctx.enter_context(tc.tile_pool(name="const", bufs=1))
        big_pool = ctx.enter_context(tc.tile_pool(name="big", bufs=1))
        w_pool = ctx.enter_context(tc.tile_pool(name="wp", bufs=10))
        cast_pool = ctx.enter_context(tc.tile_pool(name="cast", bufs=2))
        stat_pool = ctx.enter_context(tc.tile_pool(name="stat", bufs=3))
        ap_pool = ctx.enter_context(tc.tile_pool(name="appl", bufs=2))
        lr_pool = ctx.enter_context(tc.tile_pool(name="lrp", bufs=1))
        exp_pool = ctx.enter_context(tc.tile_pool(name="expp", bufs=3))
        mask_pool = ctx.enter_context(tc.tile_pool(name="maskp", bufs=2))
        bias_pool = ctx.enter_context(tc.tile_pool(name="biasp", bufs=2))
        ps_mm = ctx.enter_context(tc.tile_pool(name="psmm", bufs=2, space="PSUM"))
        ps_sc = ctx.enter_context(tc.tile_pool(name="pssc", bufs=3, space="PSUM"))
        ps_rs = ctx.enter_context(tc.tile_pool(name="psrs", bufs=2, space="PSUM"))
        ps_av = ctx.enter_context(tc.tile_pool(name="psav", bufs=1, space="PSUM"))
        if True:
            # constants
            ones_bf = const_pool.tile([128, 128], BF16, tag="ones")
            nc.vector.memset(ones_bf, 1.0)
            # half-ones for packed two-head reductions in one PSUM bank
            ones_a = const_pool.tile([128, 128], BF16, tag="ones_a")
            nc.vector.memset(ones_a, 0.0)
            nc.vector.memset(ones_a[:, 0:64], 1.0)
            ones_b = const_pool.tile([128, 128], BF16, tag="ones_b")
            nc.vector.memset(ones_b, 0.0)
            nc.vector.memset(ones_b[:, 64:128], 1.0)
            eps_t = const_pool.tile([128, 1], F32, tag="eps")
            nc.vector.memset(eps_t, EPS)

            # residual stream
            xT = res_pool.tile([128, DT, TL], F32, tag="xT")
            for dt in range(DT):
                nc.sync.dma_start(out=xT[:, dt, :], in_=x0[dt])

            # v stored zero-padded: per head 128 columns, first 64 = v,
            # last 64 = 0 -> head-pair AV matmuls can share one PSUM bank.
            vv = big_pool.tile([128, BL * 4, 16, 128], BF16, tag="vv")
            nc.vector.memset(vv[:], 0.0)

            def layernorm_chunk(dst, tck, pool_casts=False):
                """dst[:, :, tck*512:+512] <- bf16 standardized xT chunk."""
                psS = ps_mm.tile([128, 512], F32, tag="mm", name="psS")
                psQ = ps_mm.tile([128, 512], F32, tag="mm", name="psQ")
                for dt in range(DT):
                    xb = cast_pool.tile([128, 512], BF16, tag="xb", name="xb")
                    sq = cast_pool.tile([128, 512], BF16, tag="sq", name="sq")
                    if pool_casts:
                        nc.gpsimd.tensor_copy(out=xb[:], in_=xT[:, dt, ts(tck, 512)])
                        nc.gpsimd.tensor_tensor(sq[:], xT[:, dt, ts(tck, 512)],
                                                xT[:, dt, ts(tck, 512)], ALU.mult)
                    else:
                        nc.scalar.copy(out=xb[:], in_=xT[:, dt, ts(tck, 512)])
                        nc.vector.tensor_tensor(sq[:], xT[:, dt, ts(tck, 512)],
                                                xT[:, dt, ts(tck, 512)], ALU.mult)
                    nc.tensor.matmul(psS[:], ones_bf[:], xb[:],
                                     start=(dt == 0), stop=(dt == DT - 1))
                    nc.tensor.matmul(psQ[:], ones_bf[:], sq[:],
                                     start=(dt == 0), stop=(dt == DT - 1))
                # var*1024 = Q - S^2/1024  (tiles replicated over partitions)
                sc_ = stat_pool.tile([128, 512], F32, tag="st", name="sc_")
                nc.vector.tensor_copy(out=sc_[:], in_=psS[:])
                st1 = stat_pool.tile([128, 512], F32, tag="st", name="st1")
                nc.vector.tensor_tensor(st1[:], sc_[:], psS[:], ALU.mult)
                stv = stat_pool.tile([128, 512], F32, tag="st", name="stv")
                nc.vector.scalar_tensor_tensor(stv[:], st1[:], -1.0 / 1024.0,
                                               psQ[:], ALU.mult, ALU.add)
                # rstd = exp(-0.5 * ln(var + eps))
                stl = stat_pool.tile([128, 512], F32, tag="st", name="stl")
                nc.scalar.activation(out=stl[:], in_=stv[:], func=AFT.Ln,
                                     scale=1.0 / 1024.0, bias=eps_t[:])
                rstd = stat_pool.tile([128, 512], F32, tag="st", name="rstd")
                nc.scalar.activation(out=rstd[:], in_=stl[:], func=AFT.Exp,
                                     scale=-0.5)
                # mu*rstd = (S/1024) * rstd
                mr = stat_pool.tile([128, 512], F32, tag="st", name="mr")
                nc.vector.scalar_tensor_tensor(mr[:], psS[:], 1.0 / 1024.0,
                                               rstd[:], ALU.mult, ALU.mult)
                for dt in range(DT):
                    tmp = ap_pool.tile([128, 512], F32, tag="ap", name="tmp")
                    eng = nc.gpsimd if dt % 2 == 0 else nc.vector
                    eng.tensor_tensor(tmp[:], xT[:, dt, ts(tck, 512)],
                                      rstd[:], ALU.mult)
                    nc.vector.tensor_tensor(dst[:, dt, ts(tck, 512)], tmp[:],
                                            mr[:], ALU.subtract)

            for l in range(nlayers):
                # per-layer small tensors
                bqk_t = bias_pool.tile([128, 16], F32, tag="bqk")
                nc.sync.dma_start(out=bqk_t[:], in_=bqk[l])
                bout_t = bias_pool.tile([128, 8], F32, tag="bout")
                nc.sync.dma_start(out=bout_t[:], in_=bout[l])
                b1_t = bias_pool.tile([128, 32], F32, tag="b1")
                nc.sync.dma_start(out=b1_t[:], in_=b1d[l])
                b2_t = bias_pool.tile([128, 8], F32, tag="b2")
                nc.sync.dma_start(out=b2_t[:], in_=b2d[l])
                mk = []
                for b in range(BL):
                    m = mask_pool.tile([128, ACOLS], BF16, tag="mask", name=f"mk{b}")
                    nc.sync.dma_start(out=m[:], in_=bmask[b])
                    mk.append(m)

                # ---------------- LN1 ----------------
                nrm = big_pool.tile([128, DT, TL], BF16, tag="nrm")
                layernorm_chunk(nrm, 0)
                layernorm_chunk(nrm, 1)

                # ---------------- Q,K projections (tck-outer) ----------------
                qT = big_pool.tile([128, 8, TL], BF16, tag="qT")
                kT = big_pool.tile([128, 8, TL], BF16, tag="kT")
                wqk_t = []
                for g in range(4):
                    wt = w_pool.tile([128, 2, 2048], BF16, tag="W", name=f"wqk{g}")
                    nc.sync.dma_start(out=wt[:, 0], in_=wqk[l, g, :, 0])
                    nc.sync.dma_start(out=wt[:, 1], in_=wqk[l, g, :, 1])
                    wqk_t.append(wt)
                for tck in range(2):
                    for ot in range(16):
                        ps = ps_mm.tile([128, 512], F32, tag="mm", name="psqk")
                        for dt in range(DT):
                            g, s = dt // 2, dt % 2
                            nc.tensor.matmul(ps[:], wqk_t[g][:, s, ts(ot, 128)],
                                             nrm[:, dt, ts(tck, 512)],
                                             start=(dt == 0), stop=(dt == DT - 1))
                        dst = qT if ot < 8 else kT
                        nc.vector.tensor_scalar_add(
                            out=dst[:, ot % 8, ts(tck, 512)], in0=ps[:],
                            scalar1=bqk_t[:, ot:ot + 1])

                # ---------------- V (token-major, zero-padded heads) --------
                wv_t = []
                for g in range(2):
                    wt = w_pool.tile([128, 4, 1024], BF16, tag="W", name=f"wv{g}")
                    nc.sync.dma_start(out=wt[:, 0:2], in_=wv[l, g, :, 0:2])
                    nc.sync.dma_start(out=wt[:, 2:4], in_=wv[l, g, :, 2:4])
                    wv_t.append(wt)
                for tt in range(8):
                    for oc in range(2):
                        ps = ps_mm.tile([128, 512], F32, tag="mm", name="psv")
                        for dt in range(DT):
                            g, s = dt // 4, dt % 4
                            nc.tensor.matmul(ps[:], nrm[:, dt, ts(tt, 128)],
                                             wv_t[g][:, s, ts(oc, 512)],
                                             start=(dt == 0), stop=(dt == DT - 1))
                        nc.vector.tensor_copy(
                            out=vv[:, tt, ds(oc * 8, 8), 0:64],
                            in_=ps[:].rearrange("p (h e) -> p h e", e=64))

                # ---------------- attention + out-projection ----------------
                # causal structure: key chunk kt (kt>=1) only reaches queries
                # q >= 128*kt, so score/rowsum/AV matmuls skip the dead range.
                oT = big_pool.tile([128, 8, TL], BF16, tag="nrm")
                wout_t = []

                def attention_batch(b):
                    for p in range(8):  # head pair
                        et = [exp_pool.tile([128, ACOLS], BF16, tag="expt",
                                            name=f"et{h}") for h in range(2)]
                        for kt in range(4):
                            q0 = 128 * kt
                            sc = [ps_sc.tile([128, 512], F32, tag="sc",
                                             name=f"sc{h}") for h in range(2)]
                            for h in range(2):
                                pr = ds(64 * h, 64)
                                nc.tensor.matmul(
                                    sc[h][:, q0:],
                                    kT[pr, p, ds(b * T + kt * 128, 128)],
                                    qT[pr, p, ds(b * T + q0, T - q0)],
                                    start=True, stop=True)
                            for h in range(2):
                                nc.scalar.activation(
                                    out=et[h][:, ds(AOFF[kt], AW[kt])],
                                    in_=sc[h][:, q0:], func=AFT.Exp)
                        for h in range(2):
                            nc.vector.tensor_tensor(et[h][:], et[h][:], mk[b][:],
                                                    ALU.mult)
                        # row sums (heads packed in one bank via half-ones)
                        rs = ps_rs.tile([128, 512], F32, tag="rs")
                        first = True
                        for h, oh in ((0, ones_a), (1, ones_b)):
                            for kt in range(4):
                                q0 = 128 * kt
                                nc.tensor.matmul(rs[:, q0:], oh[:],
                                                 et[h][:, ds(AOFF[kt], AW[kt])],
                                                 start=first,
                                                 stop=(kt == 3 and h == 1))
                                first = False
                        lnt = stat_pool.tile([128, 512], F32, tag="st", name="lnt")
                        nc.scalar.activation(out=lnt[:], in_=rs[:], func=AFT.Ln)
                        riv = lr_pool.tile([128, 512], F32, tag="riv")
                        nc.scalar.activation(out=riv[:], in_=lnt[:], func=AFT.Exp,
                                             scale=-1.0)
                        # AV (heads packed via zero-padded v slices)
                        av = ps_av.tile([128, 512], F32, tag="av")
                        first = True
                        for h in range(2):
                            for kt in range(4):
                                q0 = 128 * kt
                                lhs = vv[:, b * 4 + kt].rearrange(
                                    "p h e -> p (h e)")[:, ds(256 * p + 64 * h, 128)]
                                nc.tensor.matmul(av[:, q0:], lhs,
                                                 et[h][:, ds(AOFF[kt], AW[kt])],
                                                 start=first,
                                                 stop=(kt == 3 and h == 1))
                                first = False
                        nc.vector.tensor_tensor(oT[:, p, ds(b * T, T)], av[:],
                                                riv[:], ALU.mult)

                def outproj_batch(b):
                    for dt in range(DT):
                        ps = ps_mm.tile([128, 512], F32, tag="mm", name="pso")
                        for ot in range(8):
                            g, s = ot // 4, ot % 4
                            nc.tensor.matmul(ps[:], wout_t[g][:, s, ts(dt, 128)],
                                             oT[:, ot, ts(b, 512)],
                                             start=(ot == 0), stop=(ot == 7))
                        nc.vector.scalar_tensor_tensor(
                            xT[:, dt, ts(b, 512)], ps[:], bout_t[:, dt:dt + 1],
                            xT[:, dt, ts(b, 512)], ALU.add, ALU.add)

                attention_batch(0)
                for g in range(2):
                    wt = w_pool.tile([128, 4, 1024], BF16, tag="W", name=f"wout{g}")
                    nc.sync.dma_start(out=wt[:, 0:2], in_=wout[l, g, :, 0:2])
                    nc.sync.dma_start(out=wt[:, 2:4], in_=wout[l, g, :, 2:4])
                    wout_t.append(wt)
                outproj_batch(0)
                attention_batch(1)
                outproj_batch(1)

                # ---------------- LN2 ----------------
                nrm2 = big_pool.tile([128, DT, TL], BF16, tag="nrm")
                layernorm_chunk(nrm2, 0)
                layernorm_chunk(nrm2, 1)

                # ---------------- FFN (f-half outer; W1/W2 loaded once) ------
                for fh in range(2):
                    w1_t = []
                    for g in range(4):
                        wt = w_pool.tile([128, 2, 2048], BF16, tag="W",
                                         name=f"w1_{g}")
                        nc.sync.dma_start(out=wt[:, 0], in_=w1[l, fh, g, :, 0])
                        nc.sync.dma_start(out=wt[:, 1], in_=w1[l, fh, g, :, 1])
                        w1_t.append(wt)
                    hts = {}
                    for tck in range(2):
                        hts[tck] = [w_pool.tile([128, 8, 512], BF16, tag="W",
                                                name=f"hts{tck}_{i}")
                                    for i in range(2)]
                        for ft in range(16):
                            ps = ps_mm.tile([128, 512], F32, tag="mm", name="psf1")
                            for dt in range(DT):
                                g, s = dt // 2, dt % 2
                                nc.tensor.matmul(ps[:], w1_t[g][:, s, ts(ft, 128)],
                                                 nrm2[:, dt, ts(tck, 512)],
                                                 start=(dt == 0), stop=(dt == DT - 1))
                            ftg = fh * 16 + ft
                            nc.scalar.activation(out=hts[tck][ft // 8][:, ft % 8, :],
                                                 in_=ps[:], func=AFT.Gelu,
                                                 bias=b1_t[:, ftg:ftg + 1])
                    w2_t = []
                    for g in range(4):
                        wt = w_pool.tile([128, 4, 1024], BF16, tag="W",
                                         name=f"w2_{g}")
                        nc.sync.dma_start(out=wt[:, 0:2], in_=w2[l, fh, g, :, 0:2])
                        nc.sync.dma_start(out=wt[:, 2:4], in_=w2[l, fh, g, :, 2:4])
                        w2_t.append(wt)
                    for tck in range(2):
                        for dt in range(DT):
                            ps = ps_mm.tile([128, 512], F32, tag="mm", name="psf2")
                            for fi in range(16):
                                g, s = fi // 4, fi % 4
                                nc.tensor.matmul(ps[:], w2_t[g][:, s, ts(dt, 128)],
                                                 hts[tck][fi // 8][:, fi % 8, :],
                                                 start=(fi == 0), stop=(fi == 15))
                            if fh == 0:
                                nc.vector.scalar_tensor_tensor(
                                    xT[:, dt, ts(tck, 512)], ps[:],
                                    b2_t[:, dt:dt + 1],
                                    xT[:, dt, ts(tck, 512)], ALU.add, ALU.add)
                            else:
                                nc.vector.tensor_tensor(
                                    xT[:, dt, ts(tck, 512)], ps[:],
                                    xT[:, dt, ts(tck, 512)], ALU.add)

            for dt in range(DT):
                nc.sync.dma_start(out=yout[dt], in_=xT[:, dt, :])

    nc.compile()
    return nc


# --------------------------------------------------------------------------
# Host-side input preparation
# --------------------------------------------------------------------------

def _combined_keep_maskT(n_text, pad):
    """Binary keep-mask, transposed: [B, k, q], 1.0 = may attend."""
    total = T
    can = np.zeros((total, total), dtype=bool)
    can[:n_text, :n_text] = True
    can[n_text:, :n_text] = True
    can[n_text:, n_text:] = np.tril(np.ones((N_MOTION, N_MOTION), dtype=bool))
    attn = ~can
    attn = np.broadcast_to(attn, (pad.shape[0], total, total)).copy()
    pad_full = np.concatenate(
        [np.zeros((pad.shape[0], n_text), dtype=bool), pad.astype(bool)], axis=1)
    attn |= pad_full[:, None, :]
    attn |= pad_full[:, :, None]
    attn &= ~np.eye(total, dtype=bool)[None]
    return (~attn).transpose(0, 2, 1).astype(np.float32)


def _prep_host(inputs):
    """Fold/transform all weights; returns dict of device arrays."""
    f = {k: np.asarray(v) for k, v in inputs.items()}
    n_text = int(f["n_text"])
    assert n_text == N_TEXT

    scale = 1.0 / math.sqrt(HD)
    dev = {}

    x = np.concatenate([f["text_tokens"], f["motion_tokens"]], axis=1)
    x = x.astype(np.float32) + f["pos_embed"][:T][None].astype(np.float32)
    dev["x_full"] = x  # [B, T, D]

    dev["keepT"] = _combined_keep_maskT(n_text, f["motion_padding_mask"])  # [B,k,q]

    wqk_l, wv_l, wout_l, w1_l, w2_l = [], [], [], [], []
    bqk_l, bout_l, b1_l, b2_l = [], [], [], []
    for l in range(L):
        g1 = f["ln1_g"][l].astype(np.float64)
        bg1 = f["ln1_b"][l].astype(np.float64)
        Wqkv = f["in_proj_w"][l].astype(np.float64)
        bqkv = f["in_proj_b"][l].astype(np.float64)
        Wq, Wk, Wv = Wqkv[:D], Wqkv[D:2 * D], Wqkv[2 * D:]
        Wq_f = Wq * g1[None, :] * scale
        Wk_f = Wk * g1[None, :]
        Wv_f = Wv * g1[None, :]
        bq_f = (bqkv[:D] + Wq @ bg1) * scale
        bk_f = bqkv[D:2 * D] + Wk @ bg1
        bv_f = bqkv[2 * D:] + Wv @ bg1
        Wo = f["out_w"][l].astype(np.float64)
        bo_f = f["out_b"][l].astype(np.float64) + Wo @ bv_f
        g2 = f["ln2_g"][l].astype(np.float64)
        bg2 = f["ln2_b"][l].astype(np.float64)
        W1 = f["ffn_w1"][l].astype(np.float64)
        W1_f = W1 * g2[None, :]
        b1_f = f["ffn_b1"][l].astype(np.float64) + W1 @ bg2
        W2 = f["ffn_w2"][l].astype(np.float64)
        b2_f = f["ffn_b2"][l].astype(np.float64)

        wqkT = np.concatenate([Wq_f, Wk_f], axis=0).T.astype(BF)  # [D, 2048]
        wqk_l.append(wqkT.reshape(4, 2, 128, 2048).transpose(0, 2, 1, 3))
        wvT = Wv_f.T.astype(BF)  # [D, 1024]
        wv_l.append(wvT.reshape(2, 4, 128, 1024).transpose(0, 2, 1, 3))
        woT = Wo.T.astype(BF)  # [o, do]
        wout_l.append(woT.reshape(2, 4, 128, 1024).transpose(0, 2, 1, 3))
        # w1T [D, DFF] -> [fh, g, 128, s, 2048]
        w1T = W1_f.T.astype(BF)
        w1_l.append(w1T.reshape(4, 2, 128, 2, 2048).transpose(3, 0, 2, 1, 4))
        # w2T [DFF, D] -> [fh, fg, 128, s, 1024]
        w2T = W2.T.astype(BF)
        w2_l.append(w2T.reshape(2, 4, 4, 128, 1024).transpose(0, 1, 3, 2, 4))

        bqk_l.append(np.concatenate([bq_f, bk_f]).astype(np.float32)
                     .reshape(16, 128).T)
        bout_l.append(bo_f.astype(np.float32).reshape(8, 128).T)
        b1_l.append(b1_f.astype(np.float32).reshape(32, 128).T)
        b2_l.append(b2_f.astype(np.float32).reshape(8, 128).T)

    dev["wqk"] = np.ascontiguousarray(np.stack(wqk_l))
    dev["wv"] = np.ascontiguousarray(np.stack(wv_l))
    dev["wout"] = np.ascontiguousarray(np.stack(wout_l))
    dev["w1"] = np.ascontiguousarray(np.stack(w1_l))
    dev["w2"] = np.ascontiguousarray(np.stack(w2_l))
    dev["bqk"] = np.ascontiguousarray(np.stack(bqk_l))
    dev["bout"] = np.ascontiguousarray(np.stack(bout_l))
    dev["b1d"] = np.ascontiguousarray(np.stack(b1_l))
    dev["b2d"] = np.ascontiguousarray(np.stack(b2_l))
    return dev


AOFF = (0, 512, 896, 1152)
AW = (512, 384, 256, 128)


def _core_in_map(dev, c):
    xc = dev["x_full"][BL * c:BL * (c + 1)]  # [BL, T, D]
    x0 = xc.transpose(2, 0, 1).reshape(D, TL).reshape(DT, 128, TL)
    km = dev["keepT"][BL * c:BL * (c + 1)]  # [BL, k, q]
    kc = km.reshape(BL, 4, 128, T)
    bm = np.zeros((BL, 128, ACOLS), dtype=BF)
    for kt in range(4):
        bm[:, :, AOFF[kt]:AOFF[kt] + AW[kt]] = kc[:, kt, :, 128 * kt:].astype(BF)
    m = {
        "x0": np.ascontiguousarray(x0.astype(np.float32)),
        "bmask": np.ascontiguousarray(bm),
    }
    for k in ("wqk", "wv", "wout", "w1", "w2", "bqk", "bout", "b1d", "b2d"):
        m[k] = dev[k]
    return m


def _assemble(results):
    out = np.empty((B, T, D), dtype=np.float32)
    for c in range(NCORES):
        y = results[c]["yout"].reshape(D, TL)  # [d, b*T+t]
        out[BL * c:BL * (c + 1)] = y.reshape(D, BL, T).transpose(1, 2, 0)
    return out


def kernel(**inputs):
    if "nc" not in _CACHE:
        _CACHE["nc"] = _build_nc()
    nc = _CACHE["nc"]
    dev = _prep_host(inputs)
    in_maps = [_core_in_map(dev, c) for c in range(NCORES)]
    res = run_bass_kernel_spmd(nc, in_maps, core_ids=list(range(NCORES)))
    _CACHE["last"] = res
    return _assemble(res.results)
